# revision 1
# baseline (speedup 1.0000x reference)
"""Multi-head causal attention on 8 Trainium2 NeuronCores.

Sharding: core c handles batch b=c//4, head group g=c%4 (4 heads of 16).
Per-core Bass kernel computes QKV projection, causal flash-style attention
(transposed-scores layout), and the out-projection partial; the host sums
the 4 per-batch partials (the out_proj all-reduce) and adds the bias.

Layout notes (per core, S=2048 tokens, D=1024, 4 heads x dh=64):
  - xT [D, S] bf16 arrives pre-transposed from host (d_in on partitions).
  - qT/kT [128, pair, S]: partitions = head-dim; pair p holds heads 2p
    (partitions 0:64) and 2p+1 (64:128).
  - scoresT tile [128 k-tokens, 512 q-tokens] = kT_slice.T @ qT_slice with
    K=dh=64 contraction: the two heads of a pair run as concurrent
    row-tiled matmuls (tile_position (0,0) and (64,0)).
  - exp: one ACT instruction per 4 score tiles (A_i,B_i,A_i+1,B_i+1 merged
    in a [128,2048] 4-bank PSUM tile) to amortize the ~352-cycle overhead.
  - ctx^T [dh, q] accumulates over k-chunks as col-tiled dual matmuls
    (head A -> psum partitions 0:64, head B -> 64:128); denominator rows
    come from parallel col-tiled ones-matmuls, replicated 64x so the
    normalization multiply is partition-aligned.
  - out^T partial [D, S] = wo.T @ ctxT, accumulated over the 2 pairs.
"""

import sys

sys.path.insert(0, "/opt/trn_rl_repo")

import numpy as np
import ml_dtypes

import concourse.bass as bass
import concourse.tile as tile
from concourse import bacc, mybir
from concourse import bass_utils

BF16 = ml_dtypes.bfloat16
F32 = mybir.dt.float32
BF = mybir.dt.bfloat16

N_CORES = 8
S = 2048          # tokens
D = 1024          # model dim
DHC = 256         # head dims per core (4 heads x 64)
DH = 64
NQ = 4            # q chunks of 512
NK = 16           # k chunks of 128
NO = 8            # d_in / d_out chunks of 128

_NC_CACHE = None


def _build_core_kernel():
    nc = bacc.Bacc("TRN2", target_bir_lowering=False, debug=False,
                   num_devices=N_CORES)
    xT = nc.dram_tensor("xT", [D, S], BF, kind="ExternalInput").ap()
    w_all = nc.dram_tensor("w_all", [D, 3 * DHC], BF, kind="ExternalInput").ap()
    wo = nc.dram_tensor("wo", [DHC, D], BF, kind="ExternalInput").ap()
    masks = nc.dram_tensor("masks", [128, 4 * 512], BF, kind="ExternalInput").ap()
    outT = nc.dram_tensor("outT", [D, S], F32, kind="ExternalOutput").ap()

    with tile.TileContext(nc) as tc:
        _emit(tc, xT, w_all, wo, masks, outT)
    nc.compile()
    return nc


def _emit(tc, xT, w_all, wo, masks, outT):
    nc = tc.nc
    EXPF = mybir.ActivationFunctionType.Exp

    from contextlib import ExitStack
    ctx = ExitStack()
    const = ctx.enter_context(tc.tile_pool(name="const", bufs=1))
    work = ctx.enter_context(tc.tile_pool(name="work", bufs=4))
    outp = ctx.enter_context(tc.tile_pool(name="outp", bufs=2))
    ps_mm = ctx.enter_context(tc.tile_pool(name="ps_mm", bufs=2, space="PSUM"))
    ps_s = ctx.enter_context(tc.tile_pool(name="ps_s", bufs=2, space="PSUM"))
    ps_c = ctx.enter_context(tc.tile_pool(name="ps_c", bufs=2, space="PSUM"))
    ps_d = ps_mm  # denominator psum is transient now; share the mm slots

    # ---- persistent SBUF tensors ----
    xt = const.tile([128, NO, S], BF, tag="xt")          # x^T, d_in chunks
    wa = const.tile([128, NO, 3 * DHC], BF, tag="wa")    # [Wq|Wk|Wv] slices
    wos = const.tile([128, 2, D], BF, tag="wos")         # Wo row chunks
    msk = const.tile([128, 4, 512], BF, tag="msk")       # causal staircases
    qt = const.tile([128, 2, S], BF, tag="qt")           # q^T per pair
    # k^T zero-padded per head so score matmuls are full-array (K=128):
    # ktpA[:, p] = [kT_head2p | 0], ktpB[:, p] = [0 | kT_head2p+1]
    ktpA = const.tile([128, 2, S], BF, tag="ktpA")
    ktpB = const.tile([128, 2, S], BF, tag="ktpB")
    # v zero-padded per head parity: even head slot = [v|0], odd = [0|v],
    # so ctx matmuls are full-array (M=128) and the two heads' outputs
    # accumulate additively in one PSUM bank.
    vsb = const.tile([128, NK, 4 * 128], BF, tag="vsb")
    ctxT = const.tile([128, 2, S], BF, tag="ctxT")       # ctx^T (normalized
    #                                                      in the post-pass)
    den_all = const.tile([128, 8, 512], F32, tag="den")  # per-chunk denoms
    ones = const.tile([128, DH], BF, tag="ones")

    def chunk_index(p, j):
        return 4 * p + j

    nc.sync.dma_start(wa[:], w_all.rearrange("(o p) f -> p o f", p=128))
    nc.sync.dma_start(msk[:], masks.rearrange("p (d f) -> p d f", f=512))
    nc.sync.dma_start(wos[:], wo.rearrange("(c p) f -> p c f", p=128))
    xTo = xT.rearrange("(o p) s -> o p s", p=128)
    for o in range(NO):  # per-chunk DMAs so matmuls start with chunk 0
        nc.sync.dma_start(xt[:, o, :], xTo[o])
    nc.vector.memset(ones[:], 1.0)
    # contiguous full-tile zero fills; data copies overwrite the live parts
    nc.gpsimd.memset(ktpA[:], 0.0)
    nc.gpsimd.memset(ktpB[:], 0.0)
    nc.gpsimd.memset(vsb[:], 0.0)

    # ---- QKV projections ----
    def emit_qk(m, streaming=False):
        # qkvT chunk m: [128 dims, S] = w_all[:, m-slice].T @ x^T
        # streaming=True: o-outer loop so work starts as x^T chunks land.
        pp = m % 2
        if streaming:
            pq0 = ps_s.tile([128, 1024], F32, tag="ps")
            pq1 = ps_s.tile([128, 1024], F32, tag="ps")
            pqs = [pq0, pq1]
            for o in range(NO):
                for n in range(NQ):
                    nc.tensor.matmul(
                        pqs[n // 2][:, 512 * (n % 2):512 * (n % 2) + 512],
                        lhsT=wa[:, o, 128 * m:128 * m + 128],
                        rhs=xt[:, o, 512 * n:512 * n + 512],
                        start=(o == 0), stop=(o == NO - 1),
                        skip_group_check=True)
        for n in range(NQ):
            n_sl = slice(512 * n, 512 * n + 512)
            if streaming:
                pq = pqs[n // 2][:, 512 * (n % 2):512 * (n % 2) + 512]
            else:
                pq = ps_mm.tile([128, 512], F32, tag="mm")
                for o in range(NO):
                    nc.tensor.matmul(
                        pq[:], lhsT=wa[:, o, 128 * m:128 * m + 128],
                        rhs=xt[:, o, n_sl],
                        start=(o == 0), stop=(o == NO - 1))
            if m < 2:
                nc.vector.tensor_copy(qt[:, pp, n_sl], pq[:])
            else:
                nc.vector.tensor_copy(ktpA[0:64, pp, n_sl], pq[0:64, :])
                nc.vector.tensor_copy(ktpB[64:128, pp, n_sl], pq[64:128, :])

    def emit_v():
        # v [tokens, 4*dh] = x @ Wv  (x^T chunks are the stationary side)
        for t in range(NK):
            pv = ps_mm.tile([128, 512], F32, tag="mm")
            for o in range(NO):
                nc.tensor.matmul(
                    pv[:, :DHC], lhsT=xt[:, o, 128 * t:128 * t + 128],
                    rhs=wa[:, o, 2 * DHC:3 * DHC],
                    start=(o == 0), stop=(o == NO - 1))
            pv4 = pv[:, :DHC].rearrange("p (h c) -> p h c", c=DH)
            dst4 = vsb.rearrange("p t (h c) -> p t h c", c=128)
            # even head slots hold [v|0], odd hold [0|v]
            nc.vector.tensor_copy(dst4[:, t, 0::2, 0:64], pv4[:, 0::2, :])
            nc.vector.tensor_copy(dst4[:, t, 1::2, 64:128], pv4[:, 1::2, :])

    # ---- attention for one (pair, q-chunk) ----
    def emit_attn_chunk(p, j):
        h0 = 2 * p
        n_i = 4 * j + 4
        q_sl = slice(512 * j, 512 * j + 512)
        pc = ps_c.tile([128, 512], F32, tag="pc")
        rs = work.tile([128, 1024], BF, tag="rs")  # per-head exp row-sums
        for i in range(n_i):
            k_sl = slice(128 * i, 128 * i + 128)
            d = i - 4 * j
            # diagonal tiles: k-chunk i only reaches q >= 128*d in this
            # q-window; restrict all work to the valid column range.
            q0 = 128 * d if d > 0 else 0
            qv_sl = slice(512 * j + q0, 512 * j + 512)
            pss = ps_s.tile([128, 1024], F32, tag="ps")
            nc.tensor.matmul(pss[:, q0:512],
                             lhsT=ktpA[:, p, k_sl], rhs=qt[:, p, qv_sl],
                             start=True, stop=True)
            nc.tensor.matmul(pss[:, 512 + q0:1024],
                             lhsT=ktpB[:, p, k_sl], rhs=qt[:, p, qv_sl],
                             start=True, stop=True)
            eT = work.tile([128, 1024], BF, tag="exp")
            if q0:
                ev = eT.rearrange("p (g f) -> p g f", g=2)[:, :, q0:512]
                pv = pss.rearrange("p (g f) -> p g f", g=2)[:, :, q0:512]
                nc.scalar.activation(ev, pv, EXPF, scale=0.125)
            else:
                nc.scalar.activation(eT[:], pss[:], EXPF, scale=0.125)
            if d >= 0:  # triangular 128x128 mask block on the diagonal
                for h in (0, 1):
                    tri = slice(512 * h + q0, 512 * h + q0 + 128)
                    nc.vector.tensor_mul(eT[:, tri], eT[:, tri],
                                         msk[:, 0, :128])
            # row-sum accumulation for the softmax denominators (bf16)
            if q0:
                rv = rs.rearrange("p (g f) -> p g f", g=2)[:, :, q0:512]
                if i == 0:
                    nc.vector.tensor_copy(rv, ev)
                else:
                    nc.vector.tensor_add(rv, rv, ev)
            elif i == 0:
                nc.vector.tensor_copy(rs[:], eT[:])
            else:
                nc.vector.tensor_add(rs[:], rs[:], eT[:])
            for h in (0, 1):  # head within pair; full-array M=128 matmuls
                sl = slice(512 * h + q0, 512 * h + 512)
                hl = h0 + h
                nc.tensor.matmul(
                    pc[:, q0:512],
                    lhsT=vsb[:, i, 128 * hl:128 * hl + 128],
                    rhs=eT[:, sl],
                    start=(i == 0 and h == 0), stop=(i == n_i - 1 and h == 1),
                    skip_group_check=True)
        # denominators: one col-tiled ones-matmul pair on the summed rows
        pd = ps_d.tile([128, 512], F32, tag="mm")
        nc.tensor.matmul(pd[0:64, :], lhsT=ones[:], rhs=rs[:, 0:512],
                         start=True, stop=True)
        nc.tensor.matmul(pd[64:128, :], lhsT=ones[:], rhs=rs[:, 512:1024],
                         start=True, stop=True)
        # stage unnormalized ctx and denominators; 1/d applied post-attention
        ci = chunk_index(p, j)
        nc.vector.tensor_copy(den_all[:, ci, :], pd[:])
        nc.vector.tensor_copy(ctxT[:, p, q_sl], pc[:])

    # ---- softmax normalization for one chunk: ctxT *= exp(-ln(den)) ----
    rec_all = const.tile([128, 8, 512], F32, tag="rec")

    def emit_normalize(p, j):
        ci = chunk_index(p, j)
        q_sl = slice(512 * j, 512 * j + 512)
        nc.scalar.activation(rec_all[:, ci, :], den_all[:, ci, :],
                             mybir.ActivationFunctionType.Ln)
        nc.scalar.activation(rec_all[:, ci, :], rec_all[:, ci, :], EXPF,
                             scale=-1.0)
        nc.vector.tensor_mul(ctxT[:, p, q_sl], ctxT[:, p, q_sl],
                             rec_all[:, ci, :])

    # ---- out projection for one token block: outT[:, n] += wo.T @ ctxT ----
    outT_m = outT.rearrange("(mm p) s -> mm p s", p=128)

    def emit_outproj_n(n):
        n_sl = slice(512 * n, 512 * n + 512)
        for m in range(NO):
            po = ps_mm.tile([128, 512], F32, tag="mm")
            for p in (0, 1):
                nc.tensor.matmul(
                    po[:], lhsT=wos[:, p, 128 * m:128 * m + 128],
                    rhs=ctxT[:, p, n_sl],
                    start=(p == 0), stop=(p == 1))
            osb = outp.tile([128, 512], F32, tag="osb")
            if m % 2 == 0:
                nc.scalar.copy(osb[:], po[:])
            else:
                nc.vector.tensor_copy(osb[:], po[:])
            nc.sync.dma_start(outT_m[m, :, n_sl], osb[:])

    emit_qk(0, streaming=True)
    emit_qk(2, streaming=True)
    emit_v()
    emit_attn_chunk(0, 3)
    emit_qk(1)
    emit_qk(3)
    emit_normalize(0, 3)
    for j in (3, 2, 1, 0):
        if j != 3:
            emit_attn_chunk(0, j)
            emit_normalize(0, j)
        emit_attn_chunk(1, j)
        emit_normalize(1, j)
        emit_outproj_n(j)
    ctx.close()


def _get_nc():
    global _NC_CACHE
    if _NC_CACHE is None:
        _NC_CACHE = _build_core_kernel()
    return _NC_CACHE


def _build_masks():
    p = np.arange(128)[:, None]
    f = np.arange(512)[None, :]
    blocks = [(128 * d + p <= f).astype(BF16) for d in range(4)]
    return np.concatenate(blocks, axis=1)


def _shard_inputs(x, Wq, Wk, Wv, Wo):
    xb = x.astype(BF16)
    masks = _build_masks()
    in_maps = []
    for c in range(N_CORES):
        b, g = divmod(c, 4)
        cols = slice(DHC * g, DHC * g + DHC)
        w_all = np.ascontiguousarray(np.concatenate(
            [Wq[:, cols], Wk[:, cols], Wv[:, cols]], axis=1).astype(BF16))
        wo_s = np.ascontiguousarray(Wo[cols, :].astype(BF16))
        xT = np.ascontiguousarray(xb[b].T)
        in_maps.append({"xT": xT, "w_all": w_all, "wo": wo_s, "masks": masks})
    return in_maps


def _unshard(results, bo):
    out = np.empty((2, S, D), np.float32)
    for b in range(2):
        acc = results[4 * b]["outT"].copy()
        for g in range(1, 4):
            acc += results[4 * b + g]["outT"]
        out[b] = acc.T + bo.astype(np.float32)
    return out


def run(x, Wq, Wk, Wv, Wo, bo, trace=False, **spmd_kwargs):
    nc = _get_nc()
    in_maps = _shard_inputs(x, Wq, Wk, Wv, Wo)
    res = bass_utils.run_bass_kernel_spmd(
        nc, in_maps, core_ids=list(range(N_CORES)), trace=trace,
        **spmd_kwargs)
    return _unshard(res.results, bo), res


def kernel(x, Wq, Wk, Wv, Wo, bo):
    out, _ = run(np.asarray(x), np.asarray(Wq), np.asarray(Wk),
                 np.asarray(Wv), np.asarray(Wo), np.asarray(bo))
    return out



# revision 3
# speedup vs baseline: 1.0531x; 1.0531x over previous
"""Multi-head causal attention on 8 Trainium2 NeuronCores.

Sharding: core c handles batch b=c//4, head group g=c%4 (4 heads of 16).
Per-core Bass kernel computes QKV projection, causal flash-style attention
(transposed-scores layout), and the out-projection partial; the host sums
the 4 per-batch partials (the out_proj all-reduce) and adds the bias.

Layout notes (per core, S=2048 tokens, D=1024, 4 heads x dh=64):
  - xT [D, S] bf16 arrives pre-transposed from host (d_in on partitions).
  - qT/kT [128, pair, S]: partitions = head-dim; pair p holds heads 2p
    (partitions 0:64) and 2p+1 (64:128).
  - scoresT tile [128 k-tokens, 512 q-tokens] = kT_slice.T @ qT_slice; k^T
    is zero-padded per head (ktpA=[kA|0], ktpB=[0|kB]) so both matmuls are
    full-array K=128 against the stacked q^T.
  - v zero-padded per head parity: even head slot = [v|0], odd = [0|v],
    so ctx matmuls are full-array (M=128) and the two heads' outputs
    accumulate additively in one PSUM bank.
  - PE pipelining: ctx matmuls for k-chunk i are emitted after the score
    matmuls of chunk i+1, so the PE never waits on the exp (scalar ACT).
  - Softmax denominators: exp row-sums accumulate on DVE (head A) and
    GpSimd (head B); one col-tiled ones-matmul pair per (pair, q-chunk),
    then DVE reciprocal_approx_fast and a fused psum->sbuf normalize mul.
    The finish (den/recip/normalize) of chunk X is emitted after chunk
    X+1's body so the PE stream never stalls at chunk boundaries (which
    would also re-throttle the PE HAM clock gate to 1.2 GHz).
  - out^T partial [D, S] = wo.T @ ctxT in bf16, DMA'd per 512-token block;
    each out-proj is deferred one attention chunk for the same reason.
"""

import sys

sys.path.insert(0, "/opt/trn_rl_repo")

import numpy as np
import ml_dtypes

import concourse.bass as bass
import concourse.tile as tile
from concourse import bacc, mybir
from concourse import bass_utils

BF16 = ml_dtypes.bfloat16
F32 = mybir.dt.float32
BF = mybir.dt.bfloat16

N_CORES = 8
S = 2048          # tokens
D = 1024          # model dim
DHC = 256         # head dims per core (4 heads x 64)
DH = 64
NQ = 4            # q chunks of 512
NK = 16           # k chunks of 128
NO = 8            # d_in / d_out chunks of 128

_NC_CACHE = None


def _build_core_kernel():
    nc = bacc.Bacc("TRN2", target_bir_lowering=False, debug=False,
                   num_devices=N_CORES)
    xT = nc.dram_tensor("xT", [D, S], BF, kind="ExternalInput").ap()
    w_all = nc.dram_tensor("w_all", [D, 3 * DHC], BF, kind="ExternalInput").ap()
    wo = nc.dram_tensor("wo", [DHC, D], BF, kind="ExternalInput").ap()
    masks = nc.dram_tensor("masks", [128, 4 * 512], BF, kind="ExternalInput").ap()
    outT = nc.dram_tensor("outT", [D, S], BF, kind="ExternalOutput").ap()

    with tile.TileContext(nc) as tc:
        _emit(tc, xT, w_all, wo, masks, outT)
    nc.compile()
    return nc


def _emit(tc, xT, w_all, wo, masks, outT):
    nc = tc.nc
    EXPF = mybir.ActivationFunctionType.Exp

    from contextlib import ExitStack
    ctx = ExitStack()
    const = ctx.enter_context(tc.tile_pool(name="const", bufs=1))
    work = ctx.enter_context(tc.tile_pool(name="work", bufs=4))
    outp = ctx.enter_context(tc.tile_pool(name="outp", bufs=2))
    nrm = ctx.enter_context(tc.tile_pool(name="nrm", bufs=2))
    ps_mm = ctx.enter_context(tc.tile_pool(name="ps_mm", bufs=2, space="PSUM"))
    ps_s = ctx.enter_context(tc.tile_pool(name="ps_s", bufs=2, space="PSUM"))
    ps_c = ctx.enter_context(tc.tile_pool(name="ps_c", bufs=2, space="PSUM"))

    # ---- persistent SBUF tensors ----
    xt = const.tile([128, NO, S], BF, tag="xt")          # x^T, d_in chunks
    wa = const.tile([128, NO, 3 * DHC], BF, tag="wa")    # [Wq|Wk|Wv] slices
    wos = const.tile([128, 2, D], BF, tag="wos")         # Wo row chunks
    msk = const.tile([128, 4, 512], BF, tag="msk")       # causal staircases
    qt = const.tile([128, 2, S], BF, tag="qt")           # q^T per pair
    # k^T zero-padded per head so score matmuls are full-array (K=128):
    # ktpA[:, p] = [kT_head2p | 0], ktpB[:, p] = [0 | kT_head2p+1]
    ktpA = const.tile([128, 2, S], BF, tag="ktpA")
    ktpB = const.tile([128, 2, S], BF, tag="ktpB")
    # v zero-padded per head parity: even head slot = [v|0], odd = [0|v]
    vsb = const.tile([128, NK, 4 * 128], BF, tag="vsb")
    ctxT = const.tile([128, 2, S], BF, tag="ctxT")       # normalized ctx^T
    ones = const.tile([128, DH], BF, tag="ones")

    def chunk_index(p, j):
        return 4 * p + j

    # per-chunk DMAs, wa/xt interleaved, so compute starts on chunk 0
    # without waiting for the full weight load
    wao = w_all.rearrange("(o p) f -> p o f", p=128)
    xTo = xT.rearrange("(o p) s -> o p s", p=128)
    for o in range(NO):
        nc.sync.dma_start(wa[:, o, :], wao[:, o, :])
        nc.sync.dma_start(xt[:, o, :], xTo[o])
    nc.sync.dma_start(msk[:], masks.rearrange("p (d f) -> p d f", f=512))
    nc.sync.dma_start(wos[:], wo.rearrange("(c p) f -> p c f", p=128))
    nc.vector.memset(ones[:], 1.0)
    # contiguous full-tile zero fills; data copies overwrite the live parts
    nc.vector.memset(ktpA[:], 0.0)
    nc.gpsimd.memset(ktpB[:], 0.0)
    nc.gpsimd.memset(vsb[:], 0.0)

    # ---- QKV projections ----
    def emit_qk(m, streaming=False):
        # qkvT chunk m: [128 dims, S] = w_all[:, m-slice].T @ x^T
        # streaming=True: o-outer loop so work starts as x^T chunks land.
        pp = m % 2
        pq0 = ps_s.tile([128, 1024], F32, tag="ps")
        pq1 = ps_s.tile([128, 1024], F32, tag="ps")
        pqs = [pq0, pq1]
        ranges = ([(o, n) for o in range(NO) for n in range(NQ)] if streaming
                  else [(o, n) for n in range(NQ) for o in range(NO)])
        for o, n in ranges:
            nc.tensor.matmul(
                pqs[n // 2][:, 512 * (n % 2):512 * (n % 2) + 512],
                lhsT=wa[:, o, 128 * m:128 * m + 128],
                rhs=xt[:, o, 512 * n:512 * n + 512],
                start=(o == 0), stop=(o == NO - 1),
                skip_group_check=True)
        for g in range(2):
            g_sl = slice(1024 * g, 1024 * g + 1024)
            if m < 2:
                nc.vector.tensor_copy(qt[:, pp, g_sl], pqs[g][:])
            else:
                nc.vector.tensor_copy(ktpA[0:64, pp, g_sl], pqs[g][0:64, :])
                nc.vector.tensor_copy(ktpB[64:128, pp, g_sl], pqs[g][64:128, :])

    def emit_v():
        # v [tokens, 4*dh] = x @ Wv  (x^T chunks are the stationary side)
        for t in range(NK):
            pv = ps_mm.tile([128, 512], F32, tag="mm")
            for o in range(NO):
                nc.tensor.matmul(
                    pv[:, :DHC], lhsT=xt[:, o, 128 * t:128 * t + 128],
                    rhs=wa[:, o, 2 * DHC:3 * DHC],
                    start=(o == 0), stop=(o == NO - 1))
            pv4 = pv[:, :DHC].rearrange("p (h c) -> p h c", c=DH)
            dst4 = vsb.rearrange("p t (h c) -> p t h c", c=128)
            # even head slots hold [v|0], odd hold [0|v]
            if t % 2 == 0:
                nc.vector.tensor_copy(dst4[:, t, 0::2, 0:64], pv4[:, 0::2, :])
                nc.vector.tensor_copy(dst4[:, t, 1::2, 64:128], pv4[:, 1::2, :])
            else:
                nc.scalar.copy(dst4[:, t, 0::2, 0:64], pv4[:, 0::2, :])
                nc.scalar.copy(dst4[:, t, 1::2, 64:128], pv4[:, 1::2, :])

    # ---- attention for one (pair, q-chunk); returns deferred finisher ----
    def emit_attn_body(p, j):
        h0 = 2 * p
        n_i = 4 * j + 4
        q_sl = slice(512 * j, 512 * j + 512)
        pc = ps_c.tile([128, 512], F32, tag="pc")
        rs = work.tile([128, 1024], BF, tag="rs")  # per-head exp row-sums

        def emit_ctx(eT, q0, i):
            for h in (0, 1):  # head within pair; full-array M=128 matmuls
                sl = slice(512 * h + q0, 512 * h + 512)
                hl = h0 + h
                nc.tensor.matmul(
                    pc[:, q0:512],
                    lhsT=vsb[:, i, 128 * hl:128 * hl + 128],
                    rhs=eT[:, sl],
                    start=(i == 0 and h == 0), stop=(i == n_i - 1 and h == 1),
                    skip_group_check=True)

        pend = None
        for i in range(n_i):
            k_sl = slice(128 * i, 128 * i + 128)
            d = i - 4 * j
            # diagonal tiles: k-chunk i only reaches q >= 128*d in this
            # q-window; restrict all work to the valid column range.
            q0 = 128 * d if d > 0 else 0
            qv_sl = slice(512 * j + q0, 512 * j + 512)
            pss = ps_s.tile([128, 1024], F32, tag="ps")
            nc.tensor.matmul(pss[:, q0:512],
                             lhsT=ktpA[:, p, k_sl], rhs=qt[:, p, qv_sl],
                             start=True, stop=True)
            nc.tensor.matmul(pss[:, 512 + q0:1024],
                             lhsT=ktpB[:, p, k_sl], rhs=qt[:, p, qv_sl],
                             start=True, stop=True)
            # previous iteration's ctx matmuls go behind this iteration's
            # scores so the PE overlaps the exp instead of stalling on it
            if pend is not None:
                emit_ctx(*pend)
            eT = work.tile([128, 1024], BF, tag="exp")
            if q0:
                ev = eT.rearrange("p (g f) -> p g f", g=2)[:, :, q0:512]
                pv_ = pss.rearrange("p (g f) -> p g f", g=2)[:, :, q0:512]
                nc.scalar.activation(ev, pv_, EXPF, scale=0.125)
            else:
                nc.scalar.activation(eT[:], pss[:], EXPF, scale=0.125)
            if d >= 0:  # triangular 128x128 mask block on the diagonal
                tri0 = slice(q0, q0 + 128)
                tri1 = slice(512 + q0, 512 + q0 + 128)
                nc.vector.tensor_mul(eT[:, tri0], eT[:, tri0], msk[:, 0, :128])
                nc.gpsimd.tensor_mul(eT[:, tri1], eT[:, tri1], msk[:, 0, :128])
            # exp row-sum accumulation: head A chain on DVE, head B on
            # GpSimd — independent chains, no cross-engine ping-pong
            a_sl = slice(q0, 512)
            b_sl = slice(512 + q0, 1024)
            if i == 0:
                nc.vector.tensor_copy(rs[:, 0:512], eT[:, 0:512])
                nc.gpsimd.tensor_copy(rs[:, 512:1024], eT[:, 512:1024])
            else:
                nc.vector.tensor_add(rs[:, a_sl], rs[:, a_sl], eT[:, a_sl])
                nc.gpsimd.tensor_add(rs[:, b_sl], rs[:, b_sl], eT[:, b_sl])
            pend = (eT, q0, i)
        emit_ctx(*pend)

        def finish():
            # denominators: one col-tiled ones-matmul pair on the summed
            # rows, then 1/den on DVE fused into the psum->sbuf normalize
            pd = ps_mm.tile([128, 512], F32, tag="mm")
            nc.tensor.matmul(pd[0:64, :], lhsT=ones[:], rhs=rs[:, 0:512],
                             start=True, stop=True)
            nc.tensor.matmul(pd[64:128, :], lhsT=ones[:], rhs=rs[:, 512:1024],
                             start=True, stop=True)
            rec = nrm.tile([128, 512], F32, tag="rec")
            nc.vector.reciprocal_approx_fast(rec[:], pd[:])
            nc.vector.tensor_mul(ctxT[:, p, q_sl], pc[:], rec[:])
        return finish

    # ---- out projection for one token block: outT[:, n] += wo.T @ ctxT ----
    outT_m = outT.rearrange("(mm p) s -> mm p s", p=128)

    def emit_outproj_n(n):
        n_sl = slice(512 * n, 512 * n + 512)
        for m in range(NO):
            po = ps_mm.tile([128, 512], F32, tag="mm")
            for p in (0, 1):
                nc.tensor.matmul(
                    po[:], lhsT=wos[:, p, 128 * m:128 * m + 128],
                    rhs=ctxT[:, p, n_sl],
                    start=(p == 0), stop=(p == 1))
            osb = outp.tile([128, 512], BF, tag="osb")
            if m % 2 == 0:
                nc.scalar.copy(osb[:], po[:])
            else:
                nc.vector.tensor_copy(osb[:], po[:])
            nc.sync.dma_start(outT_m[m, :, n_sl], osb[:])

    # Each chunk's finish (and each out-proj) is emitted one chunk late so
    # the PE queue always has independent matmul work while the scalar /
    # vector tail of the previous chunk drains.
    emit_qk(0, streaming=True)
    emit_qk(2, streaming=True)
    emit_v()
    f03 = emit_attn_body(0, 3)
    emit_qk(1)
    emit_qk(3)
    f03()
    f13 = emit_attn_body(1, 3)
    for j in (2, 1, 0):
        fa = emit_attn_body(0, j)
        f13()
        fb = emit_attn_body(1, j)
        # by now both pairs of chunk j+1 are finished and normalized
        emit_outproj_n(j + 1)
        fa()
        f13 = fb
    f13()
    emit_outproj_n(0)
    ctx.close()


def _get_nc():
    global _NC_CACHE
    if _NC_CACHE is None:
        _NC_CACHE = _build_core_kernel()
    return _NC_CACHE


def _build_masks():
    p = np.arange(128)[:, None]
    f = np.arange(512)[None, :]
    blocks = [(128 * d + p <= f).astype(BF16) for d in range(4)]
    return np.concatenate(blocks, axis=1)


def _shard_inputs(x, Wq, Wk, Wv, Wo):
    xb = x.astype(BF16)
    masks = _build_masks()
    in_maps = []
    for c in range(N_CORES):
        b, g = divmod(c, 4)
        cols = slice(DHC * g, DHC * g + DHC)
        w_all = np.ascontiguousarray(np.concatenate(
            [Wq[:, cols], Wk[:, cols], Wv[:, cols]], axis=1).astype(BF16))
        wo_s = np.ascontiguousarray(Wo[cols, :].astype(BF16))
        xT = np.ascontiguousarray(xb[b].T)
        in_maps.append({"xT": xT, "w_all": w_all, "wo": wo_s, "masks": masks})
    return in_maps


def _unshard(results, bo):
    out = np.empty((2, S, D), np.float32)
    for b in range(2):
        acc = results[4 * b]["outT"].astype(np.float32)
        for g in range(1, 4):
            acc += results[4 * b + g]["outT"].astype(np.float32)
        out[b] = acc.T + bo.astype(np.float32)
    return out


def run(x, Wq, Wk, Wv, Wo, bo, trace=False, **spmd_kwargs):
    nc = _get_nc()
    in_maps = _shard_inputs(x, Wq, Wk, Wv, Wo)
    res = bass_utils.run_bass_kernel_spmd(
        nc, in_maps, core_ids=list(range(N_CORES)), trace=trace,
        **spmd_kwargs)
    return _unshard(res.results, bo), res


def kernel(x, Wq, Wk, Wv, Wo, bo):
    out, _ = run(np.asarray(x), np.asarray(Wq), np.asarray(Wk),
                 np.asarray(Wv), np.asarray(Wo), np.asarray(bo))
    return out


# revision 6
# speedup vs baseline: 1.1831x; 1.1235x over previous
"""Multi-head causal attention on 8 Trainium2 NeuronCores.

Sharding: core c handles batch b=c//4, head group g=c%4 (4 heads of 16).
Per-core Bass kernel computes QKV projection, causal flash-style attention
(transposed-scores layout), and the out-projection partial; the host sums
the 4 per-batch partials (the out_proj all-reduce) and adds the bias.

Layout notes (per core, S=2048 tokens, D=1024, 4 heads x dh=64):
  - xT [D, S] bf16 arrives pre-transposed from host (d_in on partitions).
  - qT/kT [128, pair, S]: partitions = head-dim; pair p holds heads 2p
    (partitions 0:64) and 2p+1 (64:128); k^T zero-padded per head
    (ktpA=[kA|0], ktpB=[0|kB]) so score matmuls are full-array K=128.
  - scoresT tile [128 k-tokens, 512 q-tokens]; exp on the scalar engine
    (the only ACT table ever loaded), exp row-sums accumulate on DVE,
    diagonal causal masks as a single GpSimd multiply per k-chunk.
  - v zero-padded per head parity so ctx matmuls are full-array (M=128)
    and the two heads accumulate additively in one PSUM bank.
  - PE pipelining: ctx matmuls for k-chunk i are emitted after the score
    matmuls of chunk i+1, so the PE never waits on the exp; independent
    "filler" matmul work (pair-1 QKV projections, deferred out-proj
    blocks) is interleaved into the attention chunks so the PE stays
    busy while scalar/vector tails drain — a mostly-idle PE window
    re-throttles the PE HAM clock gate from 2.4 to 1.2 GHz.
  - Softmax denominators: one col-tiled ones-matmul pair per chunk, DVE
    reciprocal_approx_fast, then a fused psum->sbuf normalize multiply;
    each chunk's finish is emitted after the NEXT chunk's body.
  - out^T partial [D, S] f32 is DMA'd straight from PSUM (no staging).
"""

import sys

sys.path.insert(0, "/opt/trn_rl_repo")

import numpy as np
import ml_dtypes

import concourse.bass as bass
import concourse.tile as tile
from concourse import bacc, mybir
from concourse import bass_utils

BF16 = ml_dtypes.bfloat16
F32 = mybir.dt.float32
BF = mybir.dt.bfloat16

N_CORES = 8
S = 2048          # tokens
D = 1024          # model dim
DHC = 256         # head dims per core (4 heads x 64)
DH = 64
NQ = 4            # q chunks of 512
NK = 16           # k chunks of 128
NO = 8            # d_in / d_out chunks of 128

_NC_CACHE = None


def _build_core_kernel():
    nc = bacc.Bacc("TRN2", target_bir_lowering=False, debug=False,
                   num_devices=N_CORES)
    xT = nc.dram_tensor("xT", [D, S], BF, kind="ExternalInput").ap()
    w_all = nc.dram_tensor("w_all", [D, 3 * DHC], BF, kind="ExternalInput").ap()
    wo = nc.dram_tensor("wo", [DHC, D], BF, kind="ExternalInput").ap()
    masks = nc.dram_tensor("masks", [128, 4 * 512], BF, kind="ExternalInput").ap()
    outT = nc.dram_tensor("outT", [D, S], F32, kind="ExternalOutput").ap()

    with tile.TileContext(nc) as tc:
        _emit(tc, xT, w_all, wo, masks, outT)
    nc.compile()
    return nc


def _emit(tc, xT, w_all, wo, masks, outT):
    nc = tc.nc
    EXPF = mybir.ActivationFunctionType.Exp

    from contextlib import ExitStack
    ctx = ExitStack()
    const = ctx.enter_context(tc.tile_pool(name="const", bufs=1))
    work = ctx.enter_context(tc.tile_pool(name="work", bufs=4))
    outp = ctx.enter_context(tc.tile_pool(name="outp", bufs=3))
    nrm = ctx.enter_context(tc.tile_pool(name="nrm", bufs=2))
    ps_mm = ctx.enter_context(tc.tile_pool(name="ps_mm", bufs=2, space="PSUM"))
    ps_s = ctx.enter_context(tc.tile_pool(name="ps_s", bufs=2, space="PSUM"))
    ps_c = ctx.enter_context(tc.tile_pool(name="ps_c", bufs=2, space="PSUM"))

    # ---- persistent SBUF tensors ----
    xt = const.tile([128, NO, S], BF, tag="xt")          # x^T, d_in chunks
    wa = const.tile([128, NO, 3 * DHC], BF, tag="wa")    # [Wq|Wk|Wv] slices
    wos = const.tile([128, 2, D], BF, tag="wos")         # Wo row chunks
    msk = const.tile([128, 4, 512], BF, tag="msk")       # causal staircases
    qt = const.tile([128, 2, S], BF, tag="qt")           # q^T per pair
    ktpA = const.tile([128, 2, S], BF, tag="ktpA")
    ktpB = const.tile([128, 2, S], BF, tag="ktpB")
    vsb = const.tile([128, NK, 4 * 128], BF, tag="vsb")
    ctxT = const.tile([128, 2, S], BF, tag="ctxT")       # normalized ctx^T
    ones = const.tile([128, DH], BF, tag="ones")

    # per-chunk DMAs, wa/xt interleaved, so compute starts on chunk 0
    # without waiting for the full weight load
    wao = w_all.rearrange("(o p) f -> p o f", p=128)
    xTo = xT.rearrange("(o p) s -> o p s", p=128)
    for o in range(NO):
        nc.sync.dma_start(wa[:, o, :], wao[:, o, :])
        nc.sync.dma_start(xt[:, o, :], xTo[o])
    nc.sync.dma_start(msk[:], masks.rearrange("p (d f) -> p d f", f=512))
    nc.sync.dma_start(wos[:], wo.rearrange("(c p) f -> p c f", p=128))
    nc.vector.memset(ones[:], 1.0)
    nc.vector.memset(ktpA[:], 0.0)
    nc.gpsimd.memset(ktpB[:], 0.0)
    nc.gpsimd.memset(vsb[:], 0.0)
    # both heads' triangular mask as one [128, 2, 128] view for GpSimd
    msk2 = msk[:, 0, 0:256].rearrange("p (g f) -> p g f", g=2)

    # ---- QKV projections ----
    def emit_qk_stream(m):
        # qkvT chunk m: [128 dims, S] = w_all[:, m-slice].T @ x^T with the
        # o-loop outermost so work starts as x^T/w chunks land.
        pp = m % 2
        pq0 = ps_s.tile([128, 1024], F32, tag="ps")
        pq1 = ps_s.tile([128, 1024], F32, tag="ps")
        pqs = [pq0, pq1]
        for o in range(NO):
            for n in range(NQ):
                nc.tensor.matmul(
                    pqs[n // 2][:, 512 * (n % 2):512 * (n % 2) + 512],
                    lhsT=wa[:, o, 128 * m:128 * m + 128],
                    rhs=xt[:, o, 512 * n:512 * n + 512],
                    start=(o == 0), stop=(o == NO - 1),
                    skip_group_check=True)
        for g in range(2):
            g_sl = slice(1024 * g, 1024 * g + 1024)
            if m < 2:
                nc.vector.tensor_copy(qt[:, pp, g_sl], pqs[g][:])
            else:
                nc.vector.tensor_copy(ktpA[0:64, pp, g_sl], pqs[g][0:64, :])
                nc.vector.tensor_copy(ktpB[64:128, pp, g_sl], pqs[g][64:128, :])

    def qk_fillers(m):
        # pair-1 projections as 4 independent filler units (one 512-token
        # block each: 8 accumulating matmuls + a psum->sbuf copy on the
        # scalar engine, which has slack inside attention windows)
        pp = m % 2

        def make(n):
            def fill():
                n_sl = slice(512 * n, 512 * n + 512)
                pq = ps_mm.tile([128, 512], F32, tag="mm")
                for o in range(NO):
                    nc.tensor.matmul(
                        pq[:], lhsT=wa[:, o, 128 * m:128 * m + 128],
                        rhs=xt[:, o, n_sl],
                        start=(o == 0), stop=(o == NO - 1))
                if m < 2:
                    nc.scalar.copy(qt[:, pp, n_sl], pq[:])
                else:
                    nc.scalar.copy(ktpA[0:64, pp, n_sl], pq[0:64, :])
                    nc.scalar.copy(ktpB[64:128, pp, n_sl], pq[64:128, :])
            return fill
        return [make(n) for n in range(NQ)]

    def emit_v():
        # v [tokens, 4*dh] = x @ Wv  (x^T chunks are the stationary side)
        for t in range(NK):
            pv = ps_mm.tile([128, 512], F32, tag="mm")
            for o in range(NO):
                nc.tensor.matmul(
                    pv[:, :DHC], lhsT=xt[:, o, 128 * t:128 * t + 128],
                    rhs=wa[:, o, 2 * DHC:3 * DHC],
                    start=(o == 0), stop=(o == NO - 1))
            pv4 = pv[:, :DHC].rearrange("p (h c) -> p h c", c=DH)
            dst4 = vsb.rearrange("p t (h c) -> p t h c", c=128)
            # even head slots hold [v|0], odd hold [0|v]
            if t % 2 == 0:
                nc.vector.tensor_copy(dst4[:, t, 0::2, 0:64], pv4[:, 0::2, :])
                nc.vector.tensor_copy(dst4[:, t, 1::2, 64:128], pv4[:, 1::2, :])
            else:
                nc.scalar.copy(dst4[:, t, 0::2, 0:64], pv4[:, 0::2, :])
                nc.scalar.copy(dst4[:, t, 1::2, 64:128], pv4[:, 1::2, :])

    # ---- attention for one (pair, q-chunk); returns deferred finisher ----
    def emit_attn_body(p, j, fillers=()):
        h0 = 2 * p
        n_i = 4 * j + 4
        q_sl = slice(512 * j, 512 * j + 512)
        pc = ps_c.tile([128, 512], F32, tag="pc")
        rs = work.tile([128, 1024], BF, tag="rs")  # per-head exp row-sums
        fillers = list(fillers)
        spacing = max(1, n_i // (len(fillers) + 1)) if fillers else 0
        nfill = 0

        def emit_ctx(eT, q0, i):
            for h in (0, 1):  # head within pair; full-array M=128 matmuls
                sl = slice(512 * h + q0, 512 * h + 512)
                hl = h0 + h
                nc.tensor.matmul(
                    pc[:, q0:512],
                    lhsT=vsb[:, i, 128 * hl:128 * hl + 128],
                    rhs=eT[:, sl],
                    start=(i == 0 and h == 0), stop=(i == n_i - 1 and h == 1),
                    skip_group_check=True)

        pend = None
        for i in range(n_i):
            k_sl = slice(128 * i, 128 * i + 128)
            d = i - 4 * j
            # diagonal tiles: k-chunk i only reaches q >= 128*d in this
            # q-window; restrict all work to the valid column range.
            q0 = 128 * d if d > 0 else 0
            qv_sl = slice(512 * j + q0, 512 * j + 512)
            pss = ps_s.tile([128, 1024], F32, tag="ps")
            nc.tensor.matmul(pss[:, q0:512],
                             lhsT=ktpA[:, p, k_sl], rhs=qt[:, p, qv_sl],
                             start=True, stop=True)
            nc.tensor.matmul(pss[:, 512 + q0:1024],
                             lhsT=ktpB[:, p, k_sl], rhs=qt[:, p, qv_sl],
                             start=True, stop=True)
            # previous iteration's ctx matmuls go behind this iteration's
            # scores so the PE overlaps the exp instead of stalling on it
            if pend is not None:
                emit_ctx(*pend)
            if fillers and nfill < len(fillers) and i + 1 >= (nfill + 1) * spacing:
                fillers[nfill]()
                nfill += 1
            eT = work.tile([128, 1024], BF, tag="exp")
            if q0:
                ev = eT.rearrange("p (g f) -> p g f", g=2)[:, :, q0:512]
                pv_ = pss.rearrange("p (g f) -> p g f", g=2)[:, :, q0:512]
                nc.scalar.activation(ev, pv_, EXPF, scale=0.125)
            else:
                nc.scalar.activation(eT[:], pss[:], EXPF, scale=0.125)
            if d >= 0:  # both heads' triangular diagonal mask in one op
                e2 = eT.rearrange("p (g f) -> p g f", g=2)[:, :, q0:q0 + 128]
                nc.gpsimd.tensor_mul(e2, e2, msk2)
            # exp row-sum accumulation (both heads, one DVE op per k-chunk)
            if i == 0:
                nc.vector.tensor_copy(rs[:], eT[:])
            elif q0:
                rv = rs.rearrange("p (g f) -> p g f", g=2)[:, :, q0:512]
                ev2 = eT.rearrange("p (g f) -> p g f", g=2)[:, :, q0:512]
                nc.vector.tensor_add(rv, rv, ev2)
            else:
                nc.vector.tensor_add(rs[:], rs[:], eT[:])
            pend = (eT, q0, i)
        emit_ctx(*pend)
        for k in range(nfill, len(fillers)):
            fillers[k]()

        def finish():
            # denominators: one col-tiled ones-matmul pair on the summed
            # rows, then 1/den on DVE fused into the psum->sbuf normalize
            pd = ps_mm.tile([128, 512], F32, tag="mm")
            nc.tensor.matmul(pd[0:64, :], lhsT=ones[:], rhs=rs[:, 0:512],
                             start=True, stop=True)
            nc.tensor.matmul(pd[64:128, :], lhsT=ones[:], rhs=rs[:, 512:1024],
                             start=True, stop=True)
            rec = nrm.tile([128, 512], F32, tag="rec")
            nc.vector.reciprocal_approx_fast(rec[:], pd[:])
            nc.vector.tensor_mul(ctxT[:, p, q_sl], pc[:], rec[:])
        return finish

    # ---- out projection: outT[:, n] += wo.T @ ctxT ----
    outT_m = outT.rearrange("(mm p) s -> mm p s", p=128)

    def outproj_fillers(n):
        n_sl = slice(512 * n, 512 * n + 512)

        def make(m):
            def fill():
                po = ps_mm.tile([128, 512], F32, tag="mm")
                for p in (0, 1):
                    nc.tensor.matmul(
                        po[:], lhsT=wos[:, p, 128 * m:128 * m + 128],
                        rhs=ctxT[:, p, n_sl],
                        start=(p == 0), stop=(p == 1))
                osb = outp.tile([128, 512], F32, tag="osb")
                if m % 2 == 0:
                    nc.scalar.copy(osb[:], po[:])
                else:
                    nc.vector.tensor_copy(osb[:], po[:])
                nc.sync.dma_start(outT_m[m, :, n_sl], osb[:])
            return fill
        return [make(m) for m in range(NO)]

    # Chunk finishes are emitted one chunk late; out-proj j+1 and the
    # pair-1 QKV projections run as fillers inside attention bodies.
    emit_qk_stream(0)
    emit_qk_stream(2)
    emit_v()
    f03 = emit_attn_body(0, 3, fillers=qk_fillers(1) + qk_fillers(3))
    f13 = emit_attn_body(1, 3)
    f03()
    f02 = emit_attn_body(0, 2)
    f13()
    f12 = emit_attn_body(1, 2, fillers=outproj_fillers(3))
    f02()
    f01 = emit_attn_body(0, 1)
    f12()
    f11 = emit_attn_body(1, 1, fillers=outproj_fillers(2))
    f01()
    f00 = emit_attn_body(0, 0)
    f11()
    f10 = emit_attn_body(1, 0, fillers=outproj_fillers(1))
    f00()
    f10()
    for fl in outproj_fillers(0):
        fl()
    ctx.close()


def _get_nc():
    global _NC_CACHE
    if _NC_CACHE is None:
        _NC_CACHE = _build_core_kernel()
    return _NC_CACHE


def _build_masks():
    p = np.arange(128)[:, None]
    f = np.arange(512)[None, :]
    blocks = [(128 * d + p <= f).astype(BF16) for d in range(4)]
    m = np.concatenate(blocks, axis=1)
    # duplicate the d=0 triangle into cols 128:256 so the kernel can mask
    # both heads' diagonal tiles with a single [128, 2, 128] multiply
    m[:, 128:256] = m[:, 0:128]
    return m


def _shard_inputs(x, Wq, Wk, Wv, Wo):
    xb = x.astype(BF16)
    masks = _build_masks()
    in_maps = []
    for c in range(N_CORES):
        b, g = divmod(c, 4)
        cols = slice(DHC * g, DHC * g + DHC)
        w_all = np.ascontiguousarray(np.concatenate(
            [Wq[:, cols], Wk[:, cols], Wv[:, cols]], axis=1).astype(BF16))
        wo_s = np.ascontiguousarray(Wo[cols, :].astype(BF16))
        xT = np.ascontiguousarray(xb[b].T)
        in_maps.append({"xT": xT, "w_all": w_all, "wo": wo_s, "masks": masks})
    return in_maps


def _unshard(results, bo):
    out = np.empty((2, S, D), np.float32)
    for b in range(2):
        acc = results[4 * b]["outT"].astype(np.float32)
        for g in range(1, 4):
            acc += results[4 * b + g]["outT"].astype(np.float32)
        out[b] = acc.T + bo.astype(np.float32)
    return out


def run(x, Wq, Wk, Wv, Wo, bo, trace=False, **spmd_kwargs):
    nc = _get_nc()
    in_maps = _shard_inputs(x, Wq, Wk, Wv, Wo)
    res = bass_utils.run_bass_kernel_spmd(
        nc, in_maps, core_ids=list(range(N_CORES)), trace=trace,
        **spmd_kwargs)
    return _unshard(res.results, bo), res


def kernel(x, Wq, Wk, Wv, Wo, bo):
    out, _ = run(np.asarray(x), np.asarray(Wq), np.asarray(Wk),
                 np.asarray(Wv), np.asarray(Wo), np.asarray(bo))
    return out


# revision 9
# speedup vs baseline: 1.2552x; 1.0609x over previous
"""Multi-head causal attention on 8 Trainium2 NeuronCores.

Sharding: core c handles batch b=c//4, head group g=c%4 (4 heads of 16).
Per-core Bass kernel computes QKV projection, causal flash-style attention
(transposed-scores layout), and the out-projection partial; the host sums
the 4 per-batch partials (the out_proj all-reduce) and adds the bias.

Layout notes (per core, S=2048 tokens, D=1024, 4 heads x dh=64):
  - xT [D, S] bf16 arrives pre-transposed from host (d_in on partitions).
  - qT/kT [128, pair, S]: partitions = head-dim; pair p holds heads 2p
    (partitions 0:64) and 2p+1 (64:128); k^T zero-padded per head
    (ktpA=[kA|0], ktpB=[0|kB]) so score matmuls are full-array K=128.
  - scoresT tile [128 k-tokens, 512 q-tokens]; exp on the scalar engine
    (the only ACT table ever loaded), exp row-sums accumulate on DVE,
    diagonal causal masks as a single GpSimd multiply per k-chunk.
  - v zero-padded per head parity so ctx matmuls are full-array (M=128)
    and the two heads accumulate additively in one PSUM bank.
  - PE pipelining: ctx matmuls for k-chunk i are emitted after the score
    matmuls of chunk i+1, so the PE never waits on the exp; independent
    "filler" matmul work (pair-1 QKV projections, deferred out-proj
    blocks) is interleaved into the attention chunks so the PE stays
    busy while scalar/vector tails drain — a mostly-idle PE window
    re-throttles the PE HAM clock gate from 2.4 to 1.2 GHz.
  - Softmax denominators: one col-tiled ones-matmul pair per chunk, DVE
    reciprocal_approx_fast, then a fused psum->sbuf normalize multiply;
    each chunk's finish is emitted after the NEXT chunk's body.
  - out^T partial [D, S] f32 is DMA'd straight from PSUM (no staging).
"""

import sys

sys.path.insert(0, "/opt/trn_rl_repo")

import numpy as np
import ml_dtypes

import concourse.bass as bass
import concourse.tile as tile
from concourse import bacc, mybir
from concourse import bass_utils

BF16 = ml_dtypes.bfloat16
F32 = mybir.dt.float32
BF = mybir.dt.bfloat16

N_CORES = 8
S = 2048          # tokens
D = 1024          # model dim
DHC = 256         # head dims per core (4 heads x 64)
DH = 64
NQ = 4            # q chunks of 512
NK = 16           # k chunks of 128
NO = 8            # d_in / d_out chunks of 128

_NC_CACHE = None


def _build_core_kernel():
    nc = bacc.Bacc("TRN2", target_bir_lowering=False, debug=False,
                   num_devices=N_CORES)
    xT = nc.dram_tensor("xT", [D, S], BF, kind="ExternalInput").ap()
    w_all = nc.dram_tensor("w_all", [D, 3 * DHC], BF, kind="ExternalInput").ap()
    wo = nc.dram_tensor("wo", [DHC, D], BF, kind="ExternalInput").ap()
    masks = nc.dram_tensor("masks", [128, 4 * 512], BF, kind="ExternalInput").ap()
    outT = nc.dram_tensor("outT", [D, S], BF, kind="ExternalOutput").ap()

    with tile.TileContext(nc) as tc:
        _emit(tc, xT, w_all, wo, masks, outT)
    nc.compile()
    return nc


def _emit(tc, xT, w_all, wo, masks, outT):
    nc = tc.nc
    EXPF = mybir.ActivationFunctionType.Exp

    from contextlib import ExitStack
    ctx = ExitStack()
    const = ctx.enter_context(tc.tile_pool(name="const", bufs=1))
    work = ctx.enter_context(tc.tile_pool(name="work", bufs=4))
    outp = ctx.enter_context(tc.tile_pool(name="outp", bufs=3))
    nrm = ctx.enter_context(tc.tile_pool(name="nrm", bufs=2))
    ps_mm = ctx.enter_context(tc.tile_pool(name="ps_mm", bufs=2, space="PSUM"))
    ps_s = ctx.enter_context(tc.tile_pool(name="ps_s", bufs=2, space="PSUM"))
    ps_c = ctx.enter_context(tc.tile_pool(name="ps_c", bufs=2, space="PSUM"))

    # ---- persistent SBUF tensors ----
    xt = const.tile([128, NO, S], BF, tag="xt")          # x^T, d_in chunks
    wa = const.tile([128, NO, 3 * DHC], BF, tag="wa")    # [Wq|Wk|Wv] slices
    wos = const.tile([128, 2, D], BF, tag="wos")         # Wo row chunks
    msk = const.tile([128, 4, 512], BF, tag="msk")       # causal staircases
    qt = const.tile([128, 2, S], BF, tag="qt")           # q^T per pair
    ktpA = const.tile([128, 2, S], BF, tag="ktpA")
    ktpB = const.tile([128, 2, S], BF, tag="ktpB")
    vsb = const.tile([128, NK, 4 * 128], BF, tag="vsb")
    ctxT = const.tile([128, 2, S], BF, tag="ctxT")       # normalized ctx^T
    ones = const.tile([128, DH], BF, tag="ones")

    # per-chunk DMAs, wa/xt interleaved, so compute starts on chunk 0
    # without waiting for the full weight load
    wao = w_all.rearrange("(o p) f -> p o f", p=128)
    xTo = xT.rearrange("(o p) s -> o p s", p=128)
    for o in range(NO):
        nc.sync.dma_start(wa[:, o, :], wao[:, o, :])
        nc.sync.dma_start(xt[:, o, :], xTo[o])
    nc.sync.dma_start(msk[:], masks.rearrange("p (d f) -> p d f", f=512))
    nc.sync.dma_start(wos[:], wo.rearrange("(c p) f -> p c f", p=128))
    nc.vector.memset(ones[:], 1.0)
    nc.vector.memset(ktpA[:], 0.0)
    nc.gpsimd.memset(ktpB[:], 0.0)
    nc.gpsimd.memset(vsb[:], 0.0)
    # both heads' triangular mask as one [128, 2, 128] view for GpSimd
    msk2 = msk[:, 0, 0:256].rearrange("p (g f) -> p g f", g=2)

    # ---- QKV projections ----
    def emit_qk_stream(m):
        # qkvT chunk m: [128 dims, S] = w_all[:, m-slice].T @ x^T with the
        # o-loop outermost so work starts as x^T/w chunks land.
        pp = m % 2
        pq0 = ps_s.tile([128, 1024], F32, tag="ps")
        pq1 = ps_s.tile([128, 1024], F32, tag="ps")
        pqs = [pq0, pq1]
        for o in range(NO):
            for n in range(NQ):
                nc.tensor.matmul(
                    pqs[n // 2][:, 512 * (n % 2):512 * (n % 2) + 512],
                    lhsT=wa[:, o, 128 * m:128 * m + 128],
                    rhs=xt[:, o, 512 * n:512 * n + 512],
                    start=(o == 0), stop=(o == NO - 1),
                    skip_group_check=True)
        for g in range(2):
            g_sl = slice(1024 * g, 1024 * g + 1024)
            if m < 2:
                nc.vector.tensor_copy(qt[:, pp, g_sl], pqs[g][:])
            else:
                nc.vector.tensor_copy(ktpA[0:64, pp, g_sl], pqs[g][0:64, :])
                nc.vector.tensor_copy(ktpB[64:128, pp, g_sl], pqs[g][64:128, :])

    def qk_fillers(m):
        # pair-1 projections as 4 independent filler units (one 512-token
        # block each: 8 accumulating matmuls + a psum->sbuf copy on the
        # scalar engine, which has slack inside attention windows)
        pp = m % 2

        def make(n):
            def fill():
                n_sl = slice(512 * n, 512 * n + 512)
                pq = ps_mm.tile([128, 512], F32, tag="mm")
                for o in range(NO):
                    nc.tensor.matmul(
                        pq[:], lhsT=wa[:, o, 128 * m:128 * m + 128],
                        rhs=xt[:, o, n_sl],
                        start=(o == 0), stop=(o == NO - 1))
                if m < 2:
                    nc.scalar.copy(qt[:, pp, n_sl], pq[:])
                else:
                    nc.scalar.copy(ktpA[0:64, pp, n_sl], pq[0:64, :])
                    nc.scalar.copy(ktpB[64:128, pp, n_sl], pq[64:128, :])
            return fill
        return [make(n) for n in range(NQ)]

    def emit_v():
        # v [tokens, 4*dh] = x @ Wv  (x^T chunks are the stationary side)
        for t in range(NK):
            pv = ps_mm.tile([128, 512], F32, tag="mm")
            for o in range(NO):
                nc.tensor.matmul(
                    pv[:, :DHC], lhsT=xt[:, o, 128 * t:128 * t + 128],
                    rhs=wa[:, o, 2 * DHC:3 * DHC],
                    start=(o == 0), stop=(o == NO - 1))
            pv4 = pv[:, :DHC].rearrange("p (h c) -> p h c", c=DH)
            dst4 = vsb.rearrange("p t (h c) -> p t h c", c=128)
            # even head slots hold [v|0], odd hold [0|v]
            if t % 2 == 0:
                nc.vector.tensor_copy(dst4[:, t, 0::2, 0:64], pv4[:, 0::2, :])
                nc.vector.tensor_copy(dst4[:, t, 1::2, 64:128], pv4[:, 1::2, :])
            else:
                nc.scalar.copy(dst4[:, t, 0::2, 0:64], pv4[:, 0::2, :])
                nc.scalar.copy(dst4[:, t, 1::2, 64:128], pv4[:, 1::2, :])

    # ---- attention for one (pair, q-chunk); returns deferred finisher ----
    def emit_attn_body(p, j, fillers=()):
        h0 = 2 * p
        n_i = 4 * j + 4
        q_sl = slice(512 * j, 512 * j + 512)
        pc = ps_c.tile([128, 512], F32, tag="pc")
        rs = work.tile([128, 1024], BF, tag="rs")  # per-head exp row-sums
        fillers = list(fillers)
        spacing = max(1, n_i // (len(fillers) + 1)) if fillers else 0
        nfill = 0

        def emit_ctx(eT, q0, i):
            for h in (0, 1):  # head within pair; full-array M=128 matmuls
                sl = slice(512 * h + q0, 512 * h + 512)
                hl = h0 + h
                nc.tensor.matmul(
                    pc[:, q0:512],
                    lhsT=vsb[:, i, 128 * hl:128 * hl + 128],
                    rhs=eT[:, sl],
                    start=(i == 0 and h == 0), stop=(i == n_i - 1 and h == 1),
                    skip_group_check=True)

        pend = None
        for i in range(n_i):
            k_sl = slice(128 * i, 128 * i + 128)
            d = i - 4 * j
            # diagonal tiles: k-chunk i only reaches q >= 128*d in this
            # q-window; restrict all work to the valid column range.
            q0 = 128 * d if d > 0 else 0
            qv_sl = slice(512 * j + q0, 512 * j + 512)
            pss = ps_s.tile([128, 1024], F32, tag="ps")
            nc.tensor.matmul(pss[:, q0:512],
                             lhsT=ktpA[:, p, k_sl], rhs=qt[:, p, qv_sl],
                             start=True, stop=True)
            nc.tensor.matmul(pss[:, 512 + q0:1024],
                             lhsT=ktpB[:, p, k_sl], rhs=qt[:, p, qv_sl],
                             start=True, stop=True)
            # previous iteration's ctx matmuls go behind this iteration's
            # scores so the PE overlaps the exp instead of stalling on it
            if pend is not None:
                emit_ctx(*pend)
            if fillers and nfill < len(fillers) and i + 1 >= (nfill + 1) * spacing:
                fillers[nfill]()
                nfill += 1
            eT = work.tile([128, 1024], BF, tag="exp")
            if q0:
                ev = eT.rearrange("p (g f) -> p g f", g=2)[:, :, q0:512]
                pv_ = pss.rearrange("p (g f) -> p g f", g=2)[:, :, q0:512]
                nc.scalar.activation(ev, pv_, EXPF, scale=0.125)
            else:
                nc.scalar.activation(eT[:], pss[:], EXPF, scale=0.125)
            if d >= 0:  # both heads' triangular diagonal mask in one op
                e2 = eT.rearrange("p (g f) -> p g f", g=2)[:, :, q0:q0 + 128]
                nc.gpsimd.tensor_mul(e2, e2, msk2)
            # exp row-sum accumulation (both heads, one DVE op per k-chunk)
            if i == 0:
                nc.vector.tensor_copy(rs[:], eT[:])
            elif q0:
                rv = rs.rearrange("p (g f) -> p g f", g=2)[:, :, q0:512]
                ev2 = eT.rearrange("p (g f) -> p g f", g=2)[:, :, q0:512]
                nc.vector.tensor_add(rv, rv, ev2)
            else:
                nc.vector.tensor_add(rs[:], rs[:], eT[:])
            pend = (eT, q0, i)
        emit_ctx(*pend)
        for k in range(nfill, len(fillers)):
            fillers[k]()

        def finish():
            # denominators: one col-tiled ones-matmul pair on the summed
            # rows, then 1/den on DVE fused into the psum->sbuf normalize
            pd = ps_mm.tile([128, 512], F32, tag="mm")
            nc.tensor.matmul(pd[0:64, :], lhsT=ones[:], rhs=rs[:, 0:512],
                             start=True, stop=True)
            nc.tensor.matmul(pd[64:128, :], lhsT=ones[:], rhs=rs[:, 512:1024],
                             start=True, stop=True)
            rec = nrm.tile([128, 512], F32, tag="rec")
            nc.vector.reciprocal_approx_fast(rec[:], pd[:])
            nc.vector.tensor_mul(ctxT[:, p, q_sl], pc[:], rec[:])
        return finish

    # ---- out projection: outT[:, n] += wo.T @ ctxT ----
    # bf16 staging, two 128-row blocks per DMA to halve sync-queue issues
    outT_p = outT.rearrange("(mm p) s -> p mm s", p=128)

    def outproj_fillers(n):
        n_sl = slice(512 * n, 512 * n + 512)

        def make(m):  # one filler = out rows 128m .. 128(m+2)
            def fill():
                osb = outp.tile([128, 2, 512], BF, tag="osb")
                for k in (0, 1):
                    po = ps_mm.tile([128, 512], F32, tag="mm")
                    for p in (0, 1):
                        nc.tensor.matmul(
                            po[:], lhsT=wos[:, p, 128 * (m + k):128 * (m + k) + 128],
                            rhs=ctxT[:, p, n_sl],
                            start=(p == 0), stop=(p == 1))
                    if k == 0:
                        nc.scalar.copy(osb[:, 0, :], po[:])
                    else:
                        nc.vector.tensor_copy(osb[:, 1, :], po[:])
                nc.sync.dma_start(outT_p[:, m:m + 2, n_sl], osb[:])
            return fill
        return [make(m) for m in range(0, NO, 2)]

    # Chunk finishes are emitted one chunk late (the pair-1 finish rides as
    # the first filler of the next pair-0 body); out-proj j+1 and the
    # pair-1 QKV projections run as fillers inside attention bodies.
    emit_qk_stream(0)
    emit_qk_stream(2)
    emit_v()
    f03 = emit_attn_body(0, 3, fillers=qk_fillers(1) + qk_fillers(3))
    f13 = emit_attn_body(1, 3)
    f03()
    for j in (2, 1, 0):
        op = outproj_fillers(j + 1)
        if j > 0:
            fa = emit_attn_body(0, j, fillers=[f13] + op[:2])
            fb = emit_attn_body(1, j, fillers=op[2:])
        else:
            fa = emit_attn_body(0, j, fillers=[f13])
            fb = emit_attn_body(1, j, fillers=op)
        fa()
        f13 = fb
    f13()
    for fl in outproj_fillers(0):
        fl()
    ctx.close()


def _get_nc():
    global _NC_CACHE
    if _NC_CACHE is None:
        _NC_CACHE = _build_core_kernel()
    return _NC_CACHE


def _build_masks():
    p = np.arange(128)[:, None]
    f = np.arange(512)[None, :]
    blocks = [(128 * d + p <= f).astype(BF16) for d in range(4)]
    m = np.concatenate(blocks, axis=1)
    # duplicate the d=0 triangle into cols 128:256 so the kernel can mask
    # both heads' diagonal tiles with a single [128, 2, 128] multiply
    m[:, 128:256] = m[:, 0:128]
    return m


def _shard_inputs(x, Wq, Wk, Wv, Wo):
    xb = x.astype(BF16)
    masks = _build_masks()
    in_maps = []
    for c in range(N_CORES):
        b, g = divmod(c, 4)
        cols = slice(DHC * g, DHC * g + DHC)
        w_all = np.ascontiguousarray(np.concatenate(
            [Wq[:, cols], Wk[:, cols], Wv[:, cols]], axis=1).astype(BF16))
        wo_s = np.ascontiguousarray(Wo[cols, :].astype(BF16))
        xT = np.ascontiguousarray(xb[b].T)
        in_maps.append({"xT": xT, "w_all": w_all, "wo": wo_s, "masks": masks})
    return in_maps


def _unshard(results, bo):
    out = np.empty((2, S, D), np.float32)
    for b in range(2):
        acc = results[4 * b]["outT"].astype(np.float32)
        for g in range(1, 4):
            acc += results[4 * b + g]["outT"].astype(np.float32)
        out[b] = acc.T + bo.astype(np.float32)
    return out


def run(x, Wq, Wk, Wv, Wo, bo, trace=False, **spmd_kwargs):
    nc = _get_nc()
    in_maps = _shard_inputs(x, Wq, Wk, Wv, Wo)
    res = bass_utils.run_bass_kernel_spmd(
        nc, in_maps, core_ids=list(range(N_CORES)), trace=trace,
        **spmd_kwargs)
    return _unshard(res.results, bo), res


def kernel(x, Wq, Wk, Wv, Wo, bo):
    out, _ = run(np.asarray(x), np.asarray(Wq), np.asarray(Wk),
                 np.asarray(Wv), np.asarray(Wo), np.asarray(bo))
    return out
